# revision 17
# baseline (speedup 1.0000x reference)
"""Trainium2 Bass kernel: sparse (top-k) causal attention, data-parallel over batch.

Reference semantics (B=32, H=8, S=512, D=64, k_index=5):
  S_raw = (Q @ K^T) / sqrt(d_k), causal-masked
  P     = softmax(S_raw)
  rows >= k_index: keep only P >= (k_index-th largest of row)
  W     = softmax(P');  W[row 0] = 0;  out = W @ V

On-chip identities (per row):
  - no max-subtraction needed (scores ~ N(0,1))
  - top-k threshold via DVE top-8 in the exp-domain (softmax is monotone)
  - W = (E >= thr) * exp(E/Z) via one fused scalar_tensor_tensor with
    accumulated row-sum Z2; rows < k_index pass everything (thr=-1) and the
    causal-masked cols contribute exp(0)=1, matching the reference; their
    uniform tail beyond the causal tile adds (S-128) to Z2 and a rank-1
    ones @ V term to the output; row 0 is zeroed via its 1/Z2 scale.

The per-(head, q-tile) bodies are emitted software-pipelined: each engine's
instruction stream is stage-ordered across iterations so no stage's tail
blocks the next iteration's head (engines execute their streams in order).

Sharding: batch 32 -> 4 per core across 8 cores; each (b,h) independent.
Host packs Q,K pre-transposed into one [.., D, 2S] tensor and V as bf16.
"""

import math

import numpy as np
import ml_dtypes

import concourse.bass as bass
import concourse.bacc as bacc
import concourse.mybir as mybir
import concourse.tile as tile
from concourse.bass_utils import run_bass_kernel_spmd
from concourse.masks import make_causal_mask, make_identity

N_CORES = 8
F32 = mybir.dt.float32
BF16 = mybir.dt.bfloat16

# test.py hooks
TRACE = False
LAST_RESULT = None
BH_OVERRIDE = None  # dev only: limit (b,h) pairs per core
QK_DTYPE = mybir.dt.float32  # matmul1 operand dtype (f32: exact top-k selection)

_NC_CACHE = {}


def _build(bh_count: int, S: int, D: int, d_k: int, k_index: int) -> bass.Bass:
    P = 128
    NT = S // P
    KI = k_index
    NEG = -1.0e5
    scale = 1.0 / math.sqrt(float(d_k))
    assert 1 <= KI <= 8 and S % P == 0 and D <= P

    nc = bacc.Bacc("TRN2", target_bir_lowering=False, debug=False)
    qkt = nc.declare_dram_parameter("qkt", [bh_count, D, 2 * S], QK_DTYPE, isOutput=False)
    vb = nc.declare_dram_parameter("vb", [bh_count, S, D], BF16, isOutput=False)
    out = nc.declare_dram_parameter("out", [bh_count, S, D], F32, isOutput=True)

    NI = bh_count * NT  # total iterations, i -> (bh=i//NT, t=i%NT)

    with tile.TileContext(nc) as tc:
        with (
            tc.tile_pool(name="const", bufs=1) as cpool,
            tc.tile_pool(name="inp", bufs=5) as ipool,
            tc.tile_pool(name="big", bufs=6) as bpool,
            tc.tile_pool(name="wbuf", bufs=5) as wpool,
            tc.tile_pool(name="wt", bufs=6) as wtpool,
            tc.tile_pool(name="stat", bufs=24) as spool,
            tc.tile_pool(name="obuf", bufs=5) as opool,
            tc.tile_pool(name="ps_s", bufs=4, space="PSUM") as ps_s,
            tc.tile_pool(name="ps_o", bufs=4, space="PSUM") as ps_o,
        ):
            # constants
            mask_f = cpool.tile([P, P], F32)
            make_causal_mask(nc, mask_f[:, :], mask_val=NEG)
            mask_b = cpool.tile([P, P], BF16)
            nc.vector.tensor_copy(mask_b[:, :], mask_f[:, :])
            ident_f = cpool.tile([P, P], F32)
            make_identity(nc, ident_f[:, :])
            ident_b = cpool.tile([P, P], BF16)
            nc.vector.tensor_copy(ident_b[:, :], ident_f[:, :])
            ones_k = cpool.tile([P, KI], BF16)
            nc.vector.memset(ones_k[:, :], 1.0)

            st = {}  # per-iteration tile state
            bh_state = {}  # per-head tiles (qk, v, o_all)

            def s_dma(i):
                bh, t = divmod(i, NT)
                if t:
                    return
                qk_s = ipool.tile([D, 2 * S], QK_DTYPE, tag="qk", name=f"qk_{bh}")
                nc.gpsimd.dma_start(qk_s[:, :], qkt[bh])
                v_s = ipool.tile([P, NT, D], BF16, tag="v", name=f"v_{bh}")
                nc.gpsimd.dma_start(
                    v_s[:, :, :], vb[bh].rearrange("(c p) d -> p c d", p=P)
                )
                o_all = opool.tile([P, NT, D], F32, tag="o_all", name=f"oall_{bh}")
                bh_state[bh] = (qk_s, v_s, o_all)

            def s_mm1(i):
                bh, t = divmod(i, NT)
                C = P * (t + 1)
                qk_s = bh_state[bh][0]
                s_ps = ps_s.tile([P, S], F32, tag="s", name=f"sps_{i}")
                nc.tensor.matmul(
                    s_ps[:, :C],
                    lhsT=qk_s[:, bass.ts(t, P)],
                    rhs=qk_s[:, S : S + C],
                    start=True,
                    stop=False,
                )
                nc.tensor.matmul(
                    s_ps[:, bass.ts(t, P)],
                    lhsT=ident_b[:, :],
                    rhs=mask_b[:, :],
                    start=False,
                    stop=True,
                )
                st[i] = {"s_ps": s_ps}

            def s_exp1(i):
                bh, t = divmod(i, NT)
                C = P * (t + 1)
                d = st[i]
                e_s = bpool.tile([P, S], F32, tag="e", name=f"e_{i}")
                z = spool.tile([P, 1], F32, tag="z", name=f"z_{i}")
                nc.scalar.activation(
                    e_s[:, :C],
                    d["s_ps"][:, :C],
                    mybir.ActivationFunctionType.Exp,
                    scale=scale,
                    accum_out=z[:, :],
                )
                d["e"], d["z"] = e_s, z

            def s_top8(i):
                bh, t = divmod(i, NT)
                C = P * (t + 1)
                d = st[i]
                top8 = spool.tile([P, 8], F32, tag="top8", name=f"top8_{i}")
                nc.vector.max(out=top8[:, :], in_=d["e"][:, :C])
                if t == 0:
                    nc.vector.memset(top8[0:KI, KI - 1 : KI], -1.0)
                rz = spool.tile([P, 1], F32, tag="rz", name=f"rz_{i}")
                nc.vector.reciprocal(rz[:, :], d["z"][:, :])
                d["top8"], d["rz"] = top8, rz

            def s_exp2(i):
                bh, t = divmod(i, NT)
                C = P * (t + 1)
                d = st[i]
                u_s = bpool.tile([P, S], F32, tag="u", name=f"u_{i}")
                nc.scalar.activation(
                    u_s[:, :C],
                    d["e"][:, :C],
                    mybir.ActivationFunctionType.Exp,
                    scale=d["rz"][:, 0:1],
                )
                d["u"] = u_s

            def s_stt(i):
                bh, t = divmod(i, NT)
                C = P * (t + 1)
                d = st[i]
                w_s = wpool.tile([P, S], BF16, tag="w", name=f"w_{i}")
                z2 = spool.tile([P, 1], F32, tag="z2", name=f"z2_{i}")
                nc.vector.scalar_tensor_tensor(
                    out=w_s[:, :C],
                    in0=d["e"][:, :C],
                    scalar=d["top8"][:, KI - 1 : KI],
                    in1=d["u"][:, :C],
                    op0=mybir.AluOpType.is_ge,
                    op1=mybir.AluOpType.mult,
                    accum_out=z2[:, :],
                )
                if t == 0:
                    nc.vector.tensor_scalar_add(z2[0:KI, :], z2[0:KI, :], float(S - P))
                rz2 = spool.tile([P, 1], F32, tag="rz2", name=f"rz2_{i}")
                nc.vector.reciprocal(rz2[:, :], z2[:, :])
                if t == 0:
                    nc.vector.memset(rz2[0:1, :], 0.0)
                d["w"], d["rz2"] = w_s, rz2

            def s_tr(i):
                bh, t = divmod(i, NT)
                C = P * (t + 1)
                d = st[i]
                wt_s = wtpool.tile([P, NT, P], BF16, tag="wt", name=f"wt_{i}")
                nc.sync.dma_start(wt_s[:, 0 : t + 1, :], d["w"][:, :C], transpose=True)
                d["wt"] = wt_s

            def s_mm2(i):
                bh, t = divmod(i, NT)
                d = st[i]
                v_s = bh_state[bh][1]
                o_ps = ps_o.tile([P, D], F32, tag="o", name=f"ops_{i}")
                for c in range(t + 1):
                    nc.tensor.matmul(
                        o_ps[:, :],
                        lhsT=d["wt"][:, c, :],
                        rhs=v_s[:, c, :],
                        start=(c == 0),
                        stop=(c == t and t > 0),
                    )
                if t == 0:
                    for c in range(1, NT):
                        nc.tensor.matmul(
                            o_ps[0:KI, :],
                            lhsT=ones_k[:, 0:KI],
                            rhs=v_s[:, c, :],
                            start=False,
                            stop=(c == NT - 1),
                        )
                d["o_ps"] = o_ps

            def s_osc(i):
                bh, t = divmod(i, NT)
                d = st.pop(i)
                o_all = bh_state[bh][2]
                nc.vector.tensor_scalar(
                    out=o_all[:, t, :],
                    in0=d["o_ps"][:, :],
                    scalar1=d["rz2"][:, 0:1],
                    scalar2=None,
                    op0=mybir.AluOpType.mult,
                )
                if t == NT - 1:
                    nc.gpsimd.dma_start(
                        out[bh].rearrange("(c p) d -> p c d", p=P), o_all[:, :, :]
                    )
                    del bh_state[bh]

            body = [s_mm1, s_exp1, s_top8, s_exp2, s_stt, s_tr, s_mm2, s_osc]
            # G=2 head interleave, plain per-iteration emission (the Tile
            # scheduler does its own lookahead; explicit stage skewing
            # measured worse).
            G = 2
            for g0 in range(0, bh_count, G):
                members = list(range(g0, min(g0 + G, bh_count)))
                for bh in members:
                    s_dma(bh * NT)
                for t in range(NT):
                    for bh in members:
                        i = bh * NT + t
                        for fn in body:
                            fn(i)
    nc.compile()
    return nc


def _get_nc(bh_count, S, D, d_k, k_index):
    key = (bh_count, S, D, d_k, k_index, str(QK_DTYPE))
    if key not in _NC_CACHE:
        _NC_CACHE[key] = _build(bh_count, S, D, d_k, k_index)
    return _NC_CACHE[key]


def kernel(q, k, v, mask=None, d_k=None, k_index=None, **_unused):
    global LAST_RESULT
    q = np.asarray(q, dtype=np.float32)
    k = np.asarray(k, dtype=np.float32)
    v = np.asarray(v, dtype=np.float32)
    B, H, S, D = q.shape
    d_k = int(d_k) if d_k is not None else D
    k_index = int(k_index) if k_index is not None else 5

    bpc = B // N_CORES
    bh_full = bpc * H
    bh_count = BH_OVERRIDE or bh_full

    qkt = np.concatenate(
        [np.transpose(q, (0, 1, 3, 2)), np.transpose(k, (0, 1, 3, 2))], axis=3
    )  # [B, H, D, 2S]
    qkt = np.ascontiguousarray(qkt)
    vb = np.ascontiguousarray(v.astype(ml_dtypes.bfloat16))

    nc = _get_nc(bh_count, S, D, d_k, k_index)

    in_maps = []
    for i in range(N_CORES):
        sl = slice(i * bpc, (i + 1) * bpc)
        in_maps.append(
            {
                "qkt": qkt[sl].reshape(bh_full, D, 2 * S)[:bh_count],
                "vb": vb[sl].reshape(bh_full, S, D)[:bh_count],
            }
        )

    res = run_bass_kernel_spmd(
        nc, in_maps, core_ids=list(range(N_CORES)), trace=TRACE
    )
    LAST_RESULT = res

    outs = [
        np.asarray(res.results[i]["out"], dtype=np.float32) for i in range(N_CORES)
    ]
    if bh_count != bh_full:
        outs = [
            np.concatenate(
                [o, np.zeros((bh_full - bh_count, S, D), np.float32)], axis=0
            )
            for o in outs
        ]
    return np.concatenate([o.reshape(bpc, H, S, D) for o in outs], axis=0)


# revision 23
# speedup vs baseline: 1.7978x; 1.7978x over previous
"""Trainium2 Bass kernel: sparse (top-k) causal attention, data-parallel over batch.

Reference semantics (B=32, H=8, S=512, D=64, k_index=5):
  S_raw = (Q @ K^T) / sqrt(d_k), causal-masked
  P     = softmax(S_raw)
  rows >= k_index: keep only P >= (k_index-th largest of row)
  W     = softmax(P');  W[row 0] = 0;  out = W @ V

On-chip identities (per row):
  - no max-subtraction needed (scores ~ N(0,1))
  - top-k threshold via DVE top-8 in the exp-domain (softmax is monotone)
  - W = (E >= thr) * exp(E/Z) via one fused scalar_tensor_tensor with
    accumulated row-sum Z2; rows < k_index pass everything (thr=-1) and the
    causal-masked cols contribute exp(0)=1, matching the reference; their
    uniform tail beyond the causal tile adds (S-128) to Z2 and a rank-1
    ones @ V term to the output; row 0 is zeroed via its 1/Z2 scale.

The per-(head, q-tile) bodies are emitted software-pipelined: each engine's
instruction stream is stage-ordered across iterations so no stage's tail
blocks the next iteration's head (engines execute their streams in order).

Sharding: batch 32 -> 4 per core across 8 cores; each (b,h) independent.
Host packs Q,K pre-transposed into one [.., D, 2S] tensor and V as bf16.
"""

import math

import numpy as np
import ml_dtypes

import concourse.bass as bass
import concourse.bacc as bacc
import concourse.mybir as mybir
import concourse.tile as tile
from concourse.bass_utils import run_bass_kernel_spmd
from concourse.masks import make_causal_mask, make_identity

N_CORES = 8
F32 = mybir.dt.float32
BF16 = mybir.dt.bfloat16

# test.py hooks
TRACE = False
LAST_RESULT = None
BH_OVERRIDE = None  # dev only: limit (b,h) pairs per core
# matmul1 runs as 3 accumulating bf16 matmuls (qh@kh + qh@kl + ql@kh) where
# q = qh + ql, k = kh + kl are bf16 splits: ~2^-17 relative score error at
# full bf16 PE rate (fp32 matmul runs at 1/8 rate).
QK_DTYPE = BF16

_NC_CACHE = {}


def _build(bh_count: int, S: int, D: int, d_k: int, k_index: int) -> bass.Bass:
    P = 128
    NT = S // P
    KI = k_index
    NEG = -1.0e5
    scale = 1.0 / math.sqrt(float(d_k))
    assert 1 <= KI <= 8 and S % P == 0 and D <= P

    nc = bacc.Bacc("TRN2", target_bir_lowering=False, debug=False)
    qkt = nc.declare_dram_parameter("qkt", [bh_count, D, 4 * S], QK_DTYPE, isOutput=False)
    vb = nc.declare_dram_parameter("vb", [bh_count, S, D], BF16, isOutput=False)
    out = nc.declare_dram_parameter("out", [bh_count, S, D], F32, isOutput=True)

    NI = bh_count * NT  # total iterations, i -> (bh=i//NT, t=i%NT)

    with tile.TileContext(nc) as tc:
        with (
            tc.tile_pool(name="const", bufs=1) as cpool,
            tc.tile_pool(name="inp", bufs=5) as ipool,
            tc.tile_pool(name="big", bufs=6) as bpool,
            tc.tile_pool(name="wbuf", bufs=5) as wpool,
            tc.tile_pool(name="wt", bufs=6) as wtpool,
            tc.tile_pool(name="stat", bufs=24) as spool,
            tc.tile_pool(name="obuf", bufs=5) as opool,
            tc.tile_pool(name="ps_s", bufs=4, space="PSUM") as ps_s,
            tc.tile_pool(name="ps_o", bufs=4, space="PSUM") as ps_o,
        ):
            # constants
            mask_f = cpool.tile([P, P], F32)
            make_causal_mask(nc, mask_f[:, :], mask_val=NEG)
            mask_b = cpool.tile([P, P], BF16)
            nc.vector.tensor_copy(mask_b[:, :], mask_f[:, :])
            ident_f = cpool.tile([P, P], F32)
            make_identity(nc, ident_f[:, :])
            ident_b = cpool.tile([P, P], BF16)
            nc.vector.tensor_copy(ident_b[:, :], ident_f[:, :])
            ones_k = cpool.tile([P, KI], BF16)
            nc.vector.memset(ones_k[:, :], 1.0)

            st = {}  # per-iteration tile state
            bh_state = {}  # per-head tiles (qk, v, o_all)

            def s_dma(i):
                bh, t = divmod(i, NT)
                if t:
                    return
                qk_s = ipool.tile([D, 4 * S], QK_DTYPE, tag="qk", name=f"qk_{bh}")
                nc.gpsimd.dma_start(qk_s[:, :], qkt[bh])
                v_s = ipool.tile([P, NT, D], BF16, tag="v", name=f"v_{bh}")
                nc.gpsimd.dma_start(
                    v_s[:, :, :], vb[bh].rearrange("(c p) d -> p c d", p=P)
                )
                o_all = opool.tile([P, NT, D], F32, tag="o_all", name=f"oall_{bh}")
                bh_state[bh] = (qk_s, v_s, o_all)

            def s_mm1(i):
                bh, t = divmod(i, NT)
                C = P * (t + 1)
                qk_s = bh_state[bh][0]
                s_ps = ps_s.tile([P, S], F32, tag="s", name=f"sps_{i}")
                qh = qk_s[:, bass.ts(t, P)]
                ql = qk_s[:, S + t * P : S + (t + 1) * P]
                kh = qk_s[:, 2 * S : 2 * S + C]
                kl = qk_s[:, 3 * S : 3 * S + C]
                nc.tensor.matmul(s_ps[:, :C], lhsT=qh, rhs=kh, start=True, stop=False)
                nc.tensor.matmul(s_ps[:, :C], lhsT=qh, rhs=kl, start=False, stop=False)
                nc.tensor.matmul(s_ps[:, :C], lhsT=ql, rhs=kh, start=False, stop=False)
                nc.tensor.matmul(
                    s_ps[:, bass.ts(t, P)],
                    lhsT=ident_b[:, :],
                    rhs=mask_b[:, :],
                    start=False,
                    stop=True,
                )
                st[i] = {"s_ps": s_ps}

            def s_exp1(i):
                bh, t = divmod(i, NT)
                C = P * (t + 1)
                d = st[i]
                e_s = bpool.tile([P, S], F32, tag="e", name=f"e_{i}")
                z = spool.tile([P, 1], F32, tag="z", name=f"z_{i}")
                nc.scalar.activation(
                    e_s[:, :C],
                    d["s_ps"][:, :C],
                    mybir.ActivationFunctionType.Exp,
                    scale=scale,
                    accum_out=z[:, :],
                )
                d["e"], d["z"] = e_s, z

            def s_top8(i):
                bh, t = divmod(i, NT)
                C = P * (t + 1)
                d = st[i]
                top8 = spool.tile([P, 8], F32, tag="top8", name=f"top8_{i}")
                nc.vector.max(out=top8[:, :], in_=d["e"][:, :C])
                if t == 0:
                    nc.vector.memset(top8[0:KI, KI - 1 : KI], -1.0)
                rz = spool.tile([P, 1], F32, tag="rz", name=f"rz_{i}")
                nc.vector.reciprocal(rz[:, :], d["z"][:, :])
                d["top8"], d["rz"] = top8, rz

            def s_exp2(i):
                bh, t = divmod(i, NT)
                C = P * (t + 1)
                d = st[i]
                u_s = bpool.tile([P, S], F32, tag="u", name=f"u_{i}")
                nc.scalar.activation(
                    u_s[:, :C],
                    d["e"][:, :C],
                    mybir.ActivationFunctionType.Exp,
                    scale=d["rz"][:, 0:1],
                )
                d["u"] = u_s

            def s_stt(i):
                bh, t = divmod(i, NT)
                C = P * (t + 1)
                d = st[i]
                w_s = wpool.tile([P, S], BF16, tag="w", name=f"w_{i}")
                z2 = spool.tile([P, 1], F32, tag="z2", name=f"z2_{i}")
                nc.vector.scalar_tensor_tensor(
                    out=w_s[:, :C],
                    in0=d["e"][:, :C],
                    scalar=d["top8"][:, KI - 1 : KI],
                    in1=d["u"][:, :C],
                    op0=mybir.AluOpType.is_ge,
                    op1=mybir.AluOpType.mult,
                    accum_out=z2[:, :],
                )
                if t == 0:
                    nc.vector.tensor_scalar_add(z2[0:KI, :], z2[0:KI, :], float(S - P))
                rz2 = spool.tile([P, 1], F32, tag="rz2", name=f"rz2_{i}")
                nc.vector.reciprocal(rz2[:, :], z2[:, :])
                if t == 0:
                    nc.vector.memset(rz2[0:1, :], 0.0)
                d["w"], d["rz2"] = w_s, rz2

            def s_tr(i):
                bh, t = divmod(i, NT)
                C = P * (t + 1)
                d = st[i]
                wt_s = wtpool.tile([P, NT, P], BF16, tag="wt", name=f"wt_{i}")
                nc.sync.dma_start(wt_s[:, 0 : t + 1, :], d["w"][:, :C], transpose=True)
                d["wt"] = wt_s

            def s_mm2(i):
                bh, t = divmod(i, NT)
                d = st[i]
                v_s = bh_state[bh][1]
                o_ps = ps_o.tile([P, D], F32, tag="o", name=f"ops_{i}")
                for c in range(t + 1):
                    nc.tensor.matmul(
                        o_ps[:, :],
                        lhsT=d["wt"][:, c, :],
                        rhs=v_s[:, c, :],
                        start=(c == 0),
                        stop=(c == t and t > 0),
                    )
                if t == 0:
                    for c in range(1, NT):
                        nc.tensor.matmul(
                            o_ps[0:KI, :],
                            lhsT=ones_k[:, 0:KI],
                            rhs=v_s[:, c, :],
                            start=False,
                            stop=(c == NT - 1),
                        )
                d["o_ps"] = o_ps

            def s_osc(i):
                bh, t = divmod(i, NT)
                d = st.pop(i)
                o_all = bh_state[bh][2]
                nc.vector.tensor_scalar(
                    out=o_all[:, t, :],
                    in0=d["o_ps"][:, :],
                    scalar1=d["rz2"][:, 0:1],
                    scalar2=None,
                    op0=mybir.AluOpType.mult,
                )
                if t == NT - 1:
                    nc.gpsimd.dma_start(
                        out[bh].rearrange("(c p) d -> p c d", p=P), o_all[:, :, :]
                    )
                    del bh_state[bh]

            body = [s_mm1, s_exp1, s_top8, s_exp2, s_stt, s_tr, s_mm2, s_osc]
            # G=2 head interleave, plain per-iteration emission (the Tile
            # scheduler does its own lookahead; explicit stage skewing
            # measured worse).
            G = 2
            for g0 in range(0, bh_count, G):
                members = list(range(g0, min(g0 + G, bh_count)))
                for bh in members:
                    s_dma(bh * NT)
                for t in range(NT):
                    for bh in members:
                        i = bh * NT + t
                        for fn in body:
                            fn(i)
    nc.compile()
    return nc


def _get_nc(bh_count, S, D, d_k, k_index):
    key = (bh_count, S, D, d_k, k_index, str(QK_DTYPE))
    if key not in _NC_CACHE:
        _NC_CACHE[key] = _build(bh_count, S, D, d_k, k_index)
    return _NC_CACHE[key]


def kernel(q, k, v, mask=None, d_k=None, k_index=None, **_unused):
    global LAST_RESULT
    q = np.asarray(q, dtype=np.float32)
    k = np.asarray(k, dtype=np.float32)
    v = np.asarray(v, dtype=np.float32)
    B, H, S, D = q.shape
    d_k = int(d_k) if d_k is not None else D
    k_index = int(k_index) if k_index is not None else 5

    bpc = B // N_CORES
    bh_full = bpc * H
    bh_count = BH_OVERRIDE or bh_full

    qT = np.transpose(q, (0, 1, 3, 2))  # [B, H, D, S]
    kT = np.transpose(k, (0, 1, 3, 2))
    qh = qT.astype(ml_dtypes.bfloat16)
    ql = (qT - qh.astype(np.float32)).astype(ml_dtypes.bfloat16)
    kh = kT.astype(ml_dtypes.bfloat16)
    kl = (kT - kh.astype(np.float32)).astype(ml_dtypes.bfloat16)
    qkt = np.ascontiguousarray(np.concatenate([qh, ql, kh, kl], axis=3))
    vb = np.ascontiguousarray(v.astype(ml_dtypes.bfloat16))

    nc = _get_nc(bh_count, S, D, d_k, k_index)

    in_maps = []
    for i in range(N_CORES):
        sl = slice(i * bpc, (i + 1) * bpc)
        in_maps.append(
            {
                "qkt": qkt[sl].reshape(bh_full, D, 4 * S)[:bh_count],
                "vb": vb[sl].reshape(bh_full, S, D)[:bh_count],
            }
        )

    res = run_bass_kernel_spmd(
        nc, in_maps, core_ids=list(range(N_CORES)), trace=TRACE
    )
    LAST_RESULT = res

    outs = [
        np.asarray(res.results[i]["out"], dtype=np.float32) for i in range(N_CORES)
    ]
    if bh_count != bh_full:
        outs = [
            np.concatenate(
                [o, np.zeros((bh_full - bh_count, S, D), np.float32)], axis=0
            )
            for o in outs
        ]
    return np.concatenate([o.reshape(bpc, H, S, D) for o in outs], axis=0)


# revision 26
# speedup vs baseline: 3.1097x; 1.7297x over previous
"""Trainium2 Bass kernel: sparse (top-k) causal attention, data-parallel over batch.

Reference semantics (B=32, H=8, S=512, D=64, k_index=5):
  S_raw = (Q @ K^T) / sqrt(d_k), causal-masked
  P     = softmax(S_raw)
  rows >= k_index: keep only P >= (k_index-th largest of row)
  W     = softmax(P');  W[row 0] = 0;  out = W @ V

On-chip identities (per row):
  - no max-subtraction needed (scores ~ N(0,1))
  - top-k threshold via DVE top-8 in the exp-domain (softmax is monotone)
  - W = (E >= thr) * exp(E/Z) via one fused scalar_tensor_tensor with
    accumulated row-sum Z2; rows < k_index pass everything (thr=-1) and the
    causal-masked cols contribute exp(0)=1, matching the reference; their
    uniform tail beyond the causal tile adds (S-128) to Z2 and a rank-1
    ones @ V term to the output; row 0 is zeroed via its 1/Z2 scale.

The per-(head, q-tile) bodies are emitted software-pipelined: each engine's
instruction stream is stage-ordered across iterations so no stage's tail
blocks the next iteration's head (engines execute their streams in order).

Sharding: batch 32 -> 4 per core across 8 cores; each (b,h) independent.
Host packs Q,K pre-transposed into one [.., D, 2S] tensor and V as bf16.
"""

import math

import numpy as np
import ml_dtypes

import concourse.bass as bass
import concourse.bacc as bacc
import concourse.mybir as mybir
import concourse.tile as tile
from concourse.bass_utils import run_bass_kernel_spmd
from concourse.masks import make_causal_mask, make_identity

N_CORES = 8
F32 = mybir.dt.float32
BF16 = mybir.dt.bfloat16

# test.py hooks
TRACE = False
LAST_RESULT = None
BH_OVERRIDE = None  # dev only: limit (b,h) pairs per core
# matmul1 runs as 3 accumulating bf16 matmuls (qh@kh + qh@kl + ql@kh) where
# q = qh + ql, k = kh + kl are bf16 splits: ~2^-17 relative score error at
# full bf16 PE rate (fp32 matmul runs at 1/8 rate).
QK_DTYPE = BF16

_NC_CACHE = {}


def _build(bh_count: int, S: int, D: int, d_k: int, k_index: int) -> bass.Bass:
    P = 128
    NT = S // P
    KI = k_index
    NEG = -1.0e5
    scale = 1.0 / math.sqrt(float(d_k))
    assert 1 <= KI <= 8 and S % P == 0 and D <= P

    nc = bacc.Bacc("TRN2", target_bir_lowering=False, debug=False)
    qkt = nc.declare_dram_parameter("qkt", [bh_count, D, 4 * S], QK_DTYPE, isOutput=False)
    vb = nc.declare_dram_parameter("vb", [bh_count, S, D], BF16, isOutput=False)
    out = nc.declare_dram_parameter("out", [bh_count, S, D], F32, isOutput=True)

    NI = bh_count * NT  # total iterations, i -> (bh=i//NT, t=i%NT)

    with tile.TileContext(nc) as tc:
        with (
            tc.tile_pool(name="const", bufs=1) as cpool,
            tc.tile_pool(name="inp", bufs=5) as ipool,
            tc.tile_pool(name="big", bufs=6) as bpool,
            tc.tile_pool(name="wbuf", bufs=5) as wpool,
            tc.tile_pool(name="wt", bufs=6) as wtpool,
            tc.tile_pool(name="stat", bufs=24) as spool,
            tc.tile_pool(name="obuf", bufs=5) as opool,
            tc.tile_pool(name="ps_s", bufs=4, space="PSUM") as ps_s,
            tc.tile_pool(name="ps_o", bufs=2, space="PSUM") as ps_o,
            tc.tile_pool(name="ps_wt", bufs=2, space="PSUM") as ps_wt,
        ):
            # constants
            mask_f = cpool.tile([P, P], F32)
            make_causal_mask(nc, mask_f[:, :], mask_val=NEG)
            mask_b = cpool.tile([P, P], BF16)
            nc.vector.tensor_copy(mask_b[:, :], mask_f[:, :])
            ident_f = cpool.tile([P, P], F32)
            make_identity(nc, ident_f[:, :])
            ident_b = cpool.tile([P, P], BF16)
            nc.vector.tensor_copy(ident_b[:, :], ident_f[:, :])
            ones_k = cpool.tile([P, KI], BF16)
            nc.vector.memset(ones_k[:, :], 1.0)

            st = {}  # per-iteration tile state
            bh_state = {}  # per-head tiles (qk, v, o_all)

            def s_dma(i):
                bh, t = divmod(i, NT)
                if t:
                    return
                qk_s = ipool.tile([D, 4 * S], QK_DTYPE, tag="qk", name=f"qk_{bh}")
                nc.gpsimd.dma_start(qk_s[:, :], qkt[bh])
                v_s = ipool.tile([P, NT, D], BF16, tag="v", name=f"v_{bh}")
                nc.gpsimd.dma_start(
                    v_s[:, :, :], vb[bh].rearrange("(c p) d -> p c d", p=P)
                )
                o_all = opool.tile([P, NT, D], F32, tag="o_all", name=f"oall_{bh}")
                bh_state[bh] = (qk_s, v_s, o_all)

            def s_mm1(i):
                bh, t = divmod(i, NT)
                C = P * (t + 1)
                qk_s = bh_state[bh][0]
                s_ps = ps_s.tile([P, S], F32, tag="s", name=f"sps_{i}")
                qh = qk_s[:, bass.ts(t, P)]
                ql = qk_s[:, S + t * P : S + (t + 1) * P]
                kh = qk_s[:, 2 * S : 2 * S + C]
                kl = qk_s[:, 3 * S : 3 * S + C]
                nc.tensor.matmul(s_ps[:, :C], lhsT=qh, rhs=kh, start=True, stop=False)
                nc.tensor.matmul(s_ps[:, :C], lhsT=qh, rhs=kl, start=False, stop=False)
                nc.tensor.matmul(s_ps[:, :C], lhsT=ql, rhs=kh, start=False, stop=False)
                nc.tensor.matmul(s_ps[:, :C], lhsT=ql, rhs=kl, start=False, stop=False)
                nc.tensor.matmul(
                    s_ps[:, bass.ts(t, P)],
                    lhsT=ident_b[:, :],
                    rhs=mask_b[:, :],
                    start=False,
                    stop=True,
                )
                st[i] = {"s_ps": s_ps}

            def s_exp1(i):
                bh, t = divmod(i, NT)
                C = P * (t + 1)
                d = st[i]
                e_s = bpool.tile([P, S], F32, tag="e", name=f"e_{i}")
                z = spool.tile([P, 1], F32, tag="z", name=f"z_{i}")
                nc.scalar.activation(
                    e_s[:, :C],
                    d["s_ps"][:, :C],
                    mybir.ActivationFunctionType.Exp,
                    scale=scale,
                    accum_out=z[:, :],
                )
                d["e"], d["z"] = e_s, z

            def s_top8(i):
                bh, t = divmod(i, NT)
                C = P * (t + 1)
                d = st[i]
                top8 = spool.tile([P, 8], F32, tag="top8", name=f"top8_{i}")
                nc.vector.max(out=top8[:, :], in_=d["e"][:, :C])
                if t == 0:
                    nc.vector.memset(top8[0:KI, KI - 1 : KI], -1.0)
                rz = spool.tile([P, 1], F32, tag="rz", name=f"rz_{i}")
                nc.vector.reciprocal(rz[:, :], d["z"][:, :])
                d["top8"], d["rz"] = top8, rz

            def s_exp2(i):
                bh, t = divmod(i, NT)
                C = P * (t + 1)
                d = st[i]
                u_s = bpool.tile([P, S], F32, tag="u", name=f"u_{i}")
                nc.scalar.activation(
                    u_s[:, :C],
                    d["e"][:, :C],
                    mybir.ActivationFunctionType.Exp,
                    scale=d["rz"][:, 0:1],
                )
                d["u"] = u_s

            def s_stt(i):
                bh, t = divmod(i, NT)
                C = P * (t + 1)
                d = st[i]
                w_s = wpool.tile([P, S], BF16, tag="w", name=f"w_{i}")
                z2 = spool.tile([P, 1], F32, tag="z2", name=f"z2_{i}")
                nc.vector.scalar_tensor_tensor(
                    out=w_s[:, :C],
                    in0=d["e"][:, :C],
                    scalar=d["top8"][:, KI - 1 : KI],
                    in1=d["u"][:, :C],
                    op0=mybir.AluOpType.is_ge,
                    op1=mybir.AluOpType.mult,
                    accum_out=z2[:, :],
                )
                if t == 0:
                    nc.vector.tensor_scalar_add(z2[0:KI, :], z2[0:KI, :], float(S - P))
                rz2 = spool.tile([P, 1], F32, tag="rz2", name=f"rz2_{i}")
                nc.vector.reciprocal(rz2[:, :], z2[:, :])
                if t == 0:
                    nc.vector.memset(rz2[0:1, :], 0.0)
                d["w"], d["rz2"] = w_s, rz2

            def s_tr(i):
                bh, t = divmod(i, NT)
                C = P * (t + 1)
                d = st[i]
                wtp = ps_wt.tile([P, NT, P], BF16, tag="wtp", name=f"wtp_{i}")
                for c in range(t + 1):
                    nc.tensor.transpose(
                        wtp[:, c, :], d["w"][:, bass.ts(c, P)], ident_b[:, :]
                    )
                wt_s = wtpool.tile([P, NT, P], BF16, tag="wt", name=f"wt_{i}")
                if i % 2 == 0:
                    nc.vector.tensor_copy(wt_s[:, 0 : t + 1, :], wtp[:, 0 : t + 1, :])
                else:
                    nc.scalar.copy(wt_s[:, 0 : t + 1, :], wtp[:, 0 : t + 1, :])
                d["wt"] = wt_s

            def s_mm2(i):
                bh, t = divmod(i, NT)
                d = st[i]
                v_s = bh_state[bh][1]
                o_ps = ps_o.tile([P, D], F32, tag="o", name=f"ops_{i}")
                for c in range(t + 1):
                    nc.tensor.matmul(
                        o_ps[:, :],
                        lhsT=d["wt"][:, c, :],
                        rhs=v_s[:, c, :],
                        start=(c == 0),
                        stop=(c == t and t > 0),
                    )
                if t == 0:
                    for c in range(1, NT):
                        nc.tensor.matmul(
                            o_ps[0:KI, :],
                            lhsT=ones_k[:, 0:KI],
                            rhs=v_s[:, c, :],
                            start=False,
                            stop=(c == NT - 1),
                        )
                d["o_ps"] = o_ps

            def s_osc(i):
                bh, t = divmod(i, NT)
                d = st.pop(i)
                o_all = bh_state[bh][2]
                nc.vector.tensor_scalar(
                    out=o_all[:, t, :],
                    in0=d["o_ps"][:, :],
                    scalar1=d["rz2"][:, 0:1],
                    scalar2=None,
                    op0=mybir.AluOpType.mult,
                )
                if t == NT - 1:
                    nc.gpsimd.dma_start(
                        out[bh].rearrange("(c p) d -> p c d", p=P), o_all[:, :, :]
                    )
                    del bh_state[bh]

            body = [s_mm1, s_exp1, s_top8, s_exp2, s_stt, s_tr, s_mm2, s_osc]
            # G=2 head interleave, plain per-iteration emission (the Tile
            # scheduler does its own lookahead; explicit stage skewing
            # measured worse).
            G = 2
            for g0 in range(0, bh_count, G):
                members = list(range(g0, min(g0 + G, bh_count)))
                for bh in members:
                    s_dma(bh * NT)
                for t in range(NT):
                    for bh in members:
                        i = bh * NT + t
                        for fn in body:
                            fn(i)
    nc.compile()
    return nc


def _get_nc(bh_count, S, D, d_k, k_index):
    key = (bh_count, S, D, d_k, k_index, str(QK_DTYPE))
    if key not in _NC_CACHE:
        _NC_CACHE[key] = _build(bh_count, S, D, d_k, k_index)
    return _NC_CACHE[key]


def kernel(q, k, v, mask=None, d_k=None, k_index=None, **_unused):
    global LAST_RESULT
    q = np.asarray(q, dtype=np.float32)
    k = np.asarray(k, dtype=np.float32)
    v = np.asarray(v, dtype=np.float32)
    B, H, S, D = q.shape
    d_k = int(d_k) if d_k is not None else D
    k_index = int(k_index) if k_index is not None else 5

    bpc = B // N_CORES
    bh_full = bpc * H
    bh_count = BH_OVERRIDE or bh_full

    qT = np.transpose(q, (0, 1, 3, 2))  # [B, H, D, S]
    kT = np.transpose(k, (0, 1, 3, 2))
    qh = qT.astype(ml_dtypes.bfloat16)
    ql = (qT - qh.astype(np.float32)).astype(ml_dtypes.bfloat16)
    kh = kT.astype(ml_dtypes.bfloat16)
    kl = (kT - kh.astype(np.float32)).astype(ml_dtypes.bfloat16)
    qkt = np.ascontiguousarray(np.concatenate([qh, ql, kh, kl], axis=3))
    vb = np.ascontiguousarray(v.astype(ml_dtypes.bfloat16))

    nc = _get_nc(bh_count, S, D, d_k, k_index)

    in_maps = []
    for i in range(N_CORES):
        sl = slice(i * bpc, (i + 1) * bpc)
        in_maps.append(
            {
                "qkt": qkt[sl].reshape(bh_full, D, 4 * S)[:bh_count],
                "vb": vb[sl].reshape(bh_full, S, D)[:bh_count],
            }
        )

    res = run_bass_kernel_spmd(
        nc, in_maps, core_ids=list(range(N_CORES)), trace=TRACE
    )
    LAST_RESULT = res

    outs = [
        np.asarray(res.results[i]["out"], dtype=np.float32) for i in range(N_CORES)
    ]
    if bh_count != bh_full:
        outs = [
            np.concatenate(
                [o, np.zeros((bh_full - bh_count, S, D), np.float32)], axis=0
            )
            for o in outs
        ]
    return np.concatenate([o.reshape(bpc, H, S, D) for o in outs], axis=0)
